# revision 15
# baseline (speedup 1.0000x reference)
"""Trainium2 Bass kernel for single-query attention over per-sample concepts.

    sab[b, k] = (query[b] . concept[b, k]) / sqrt(D)
    score     = softmax(sab, axis=-1)
    out[b]    = sum_k score[b, k] * concept[b, k]

Shapes: query [256, 1024] f32, concept [256, 2048, 1024] f32 -> out [256, 1024].

Sharding: pure data parallel, batch 256 split as 32 samples on each of 8
NeuronCores. Memory-bound: each core streams its 256 MiB concept shard once.

v3 dataflow (vs the 512x512KiB-DMA baseline):
  - concept viewed as [32 samples, 8 units, 128 partitions, 2048 floats]
    (1 MiB units). Mid-stream samples DMA 4 units at once (4 MiB per
    dma_start, 32 KiB contiguous per partition line) alternating the
    SP/ACT HWDGE rings - large sequential reads lift HBM efficiency under
    pair-core contention (~325 -> ~345 GB/s shared phase). First/last
    sample use 1 MiB granules so DVE compute overlaps the stream's ramp
    and tail.
  - a granule's partition p holds `span*2` consecutive k rows; per
    1024-float sub-column j: DVE scalar_tensor_tensor (c*scale)*qb with
    accum_out -> raw score column, ACT exp, then two PE matmuls
    (e_t.T @ c cols) into per-sample PSUM accumulators (softmax order
    over k is irrelevant).
  - per-sample PSUM accumulators [1, 512] x2 (PE requires matmul output
    base partition in {0, 32, 64}); denominator via ones-stationary matmul,
    DVE reciprocal, scaled ACT copies, per-row store (131 KiB total).
  - per-sample query row DMA + GPSIMD partition_broadcast (tiny).
"""

import numpy as np
from contextlib import ExitStack

import concourse.bacc as bacc
import concourse.tile as tile
from concourse import mybir
from concourse.bass_utils import run_bass_kernel_spmd

B, K, D = 256, 2048, 1024
NCORES = 8
BL = B // NCORES          # 32 samples per core
NU = 8                    # 1 MiB units per sample (256 k-rows each)
UF = D * K // NU // 128   # 2048 floats per partition per unit
NT = K // 128             # 16 e-columns per sample
SCALE = 1.0 / float(np.sqrt(D))

_cache = {}


def build_nc():
    nc = bacc.Bacc("TRN2", target_bir_lowering=False, debug=False,
                   num_devices=NCORES)
    q = nc.dram_tensor("query", [BL, D], mybir.dt.float32, kind="ExternalInput")
    c = nc.dram_tensor("concept", [BL, NU, 128, UF], mybir.dt.float32r,
                       kind="ExternalInput")
    out = nc.dram_tensor("out", [BL, D], mybir.dt.float32,
                         kind="ExternalOutput")
    f32 = mybir.dt.float32
    f32r = mybir.dt.float32r

    with tile.TileContext(nc) as tc, ExitStack() as ctx:
        cpool = ctx.enter_context(tc.tile_pool(name="c", bufs=5))
        qpool = ctx.enter_context(tc.tile_pool(name="q", bufs=2))
        spool = ctx.enter_context(tc.tile_pool(name="scr", bufs=2))
        epool = ctx.enter_context(tc.tile_pool(name="e", bufs=3))
        onepool = ctx.enter_context(tc.tile_pool(name="one", bufs=1))
        opool = ctx.enter_context(tc.tile_pool(name="o", bufs=3))
        ppool = ctx.enter_context(tc.tile_pool(name="ps", bufs=2, space="PSUM"))
        dpool = ctx.enter_context(tc.tile_pool(name="dn", bufs=2, space="PSUM"))

        ones = onepool.tile([128, 1], f32)
        nc.vector.memset(ones[:], 1.0)

        for b in range(BL):
            qrow = qpool.tile([1, D], f32)
            nc.scalar.dma_start(out=qrow[:], in_=q[b : b + 1, :])
            qb = qpool.tile([128, D], f32)
            nc.gpsimd.partition_broadcast(qb[:], qrow[:])

            scols = epool.tile([128, NT], f32)
            ecols = epool.tile([128, NT], f32r)
            acc_lo = ppool.tile([1, 512], f32)
            acc_hi = ppool.tile([1, 512], f32)

            # granule plan: 4 MiB DMAs mid-stream for HBM efficiency;
            # 1 MiB DMAs for the first/last sample so compute overlaps
            # the stream's ramp and tail.
            span = 1 if b in (0, BL - 1) else 4
            t = 0
            for g in range(NU // span):
                u0 = g * span
                ct = cpool.tile([128, span * UF], f32r)
                dma_eng = nc.sync if (b * NU + u0) % (2 * span) == 0 else nc.scalar
                dma_eng.dma_start(out=ct[:], in_=c[b, u0 : u0 + span])
                for j in range(2 * span):
                    scr = spool.tile([128, D], f32)
                    nc.vector.scalar_tensor_tensor(
                        out=scr[:],
                        in0=ct[:, j * D : (j + 1) * D].bitcast(f32),
                        scalar=SCALE,
                        in1=qb[:],
                        op0=mybir.AluOpType.mult,
                        op1=mybir.AluOpType.mult,
                        accum_out=scols[:, t : t + 1],
                    )
                    nc.scalar.activation(
                        out=ecols[:, t : t + 1],
                        in_=scols[:, t : t + 1],
                        func=mybir.ActivationFunctionType.Exp,
                    )
                    e_t = ecols[:, t : t + 1]
                    nc.tensor.matmul(acc_lo[:], e_t,
                                     ct[:, j * D : j * D + 512],
                                     start=(t == 0), stop=(t == NT - 1))
                    nc.tensor.matmul(acc_hi[:], e_t,
                                     ct[:, j * D + 512 : (j + 1) * D],
                                     start=(t == 0), stop=(t == NT - 1))
                    t += 1

            # denominator: per-partition sums of e, then reduce across
            # partitions with a ones-stationary matmul
            ered = epool.tile([128, 1], f32)
            escr = epool.tile([128, NT], f32)
            nc.scalar.activation(
                out=escr[:],
                in_=ecols[:].bitcast(f32),
                func=mybir.ActivationFunctionType.Copy,
                accum_out=ered[:],
            )
            denom = dpool.tile([1, 1], f32)
            nc.tensor.matmul(denom[:], ones[:], ered[:], start=True, stop=True)

            recip = opool.tile([1, 1], f32)
            nc.vector.reciprocal(recip[:], denom[:])
            orow = opool.tile([1, D], f32)
            nc.scalar.activation(out=orow[:, 0:512], in_=acc_lo[:],
                                 func=mybir.ActivationFunctionType.Copy,
                                 scale=recip[:])
            nc.scalar.activation(out=orow[:, 512:1024], in_=acc_hi[:],
                                 func=mybir.ActivationFunctionType.Copy,
                                 scale=recip[:])
            nc.scalar.dma_start(out=out[b : b + 1, :], in_=orow[:])

    nc.compile()
    return nc


def _run(query, concept, trace=False, trace_kwargs=None):
    if "nc" not in _cache:
        _cache["nc"] = build_nc()
    nc = _cache["nc"]
    in_maps = []
    for i in range(NCORES):
        cshard = np.ascontiguousarray(concept[i * BL : (i + 1) * BL])
        in_maps.append({
            "query": np.ascontiguousarray(query[i * BL : (i + 1) * BL]),
            "concept": cshard.reshape(BL, NU, 128, UF),
        })
    res = run_bass_kernel_spmd(
        nc, in_maps, core_ids=list(range(NCORES)),
        trace=trace, **(trace_kwargs or {}),
    )
    out = np.concatenate([res.results[i]["out"] for i in range(NCORES)], axis=0)
    return out.astype(np.float32), res


def kernel(query: np.ndarray, concept: np.ndarray) -> np.ndarray:
    out, _ = _run(np.asarray(query, np.float32), np.asarray(concept, np.float32))
    return out
